# revision 23
# baseline (speedup 1.0000x reference)
"""CapsNet routing layer (nn_CapsLayer) on 8 Trainium2 NeuronCores.

reference:
    u_hat = einsum("ncoi,bci->bnco", W[0], x)         # B,N,C,O = 1024,2,512,64
    3 dynamic-routing iterations (softmax over n, weighted sum over c,
    squash, agreement update); returns v from iteration 3.

Strategy (v2 — fp16 fast-mode rewrite of the fp32/bf16 baseline):
  - 8 cores x 64 in-caps, every core sees the full batch (B in 8 chunks
    of 128 = partition dim). W replicated per c-shard (4 MiB fp16).
  - Single-term fp16 GEMM (fp16 x/W, fp32 PSUM): u_hat rel err ~5e-4,
    which survives routing amplification at ~2.7e-3 final (gate 2e-2).
    bf16 would land at 1.7e-2 — no margin.
  - u_hat stored fp16 in SBUF, single layout [b, n, o, c] ("OC").
    All big DVE ops are InstTensorScalarPtr (scalar_tensor_tensor) with
    2-byte packed stride-1 operands in SBUF => 4x_2p mode (0.26 ns/elem
    vs 1.04 at 1x). InstTensorReduce has NO fast modes, so reductions
    are pairwise-halving STT trees (last 2 stages fp32) instead.
  - s-pass: product u*coeff via STT (coeff [b,n,c] broadcast over o =
    middle dim, innermost stays stride-1), tree over innermost c.
  - y-pass: product u*v_mat where v_mat = v broadcast over c,
    materialized by the ACT engine (no stride penalty there); tree over
    o = middle dim (STT stages only check innermost stride).
  - squash almost entirely on ACT: Square with accum_out gives |s|^2
    row sums; v = s * mmv via per-partition activation scale.
  - Cross-core reduction of s: AllReduces batched per chunk-group
    (groups of 3/3/2 chunks) => 9 collectives instead of 24; groups are
    interleaved in emission order so one group's AR hides behind the
    other group's DVE/ACT work (engines are in-order; emission order is
    the schedule). AR outputs use addr_space="Shared" (fast HBM-HBM
    path).
"""
import os
import sys
import types

sys.path.insert(0, "/opt/trn_rl_repo")

import numpy as np
import concourse.bass as bass
import concourse.mybir as mybir
import concourse.tile as tile
from concourse.bass_utils import run_bass_kernel_spmd

B, NCAPS, C, ICH, OCH = 1024, 2, 512, 256, 64
ITERATIONS = 3
NCORES = 8
CPC = C // NCORES            # in-caps per core = 64
NBCH = 8                     # batch chunks
BCH = B // NBCH              # samples per chunk = 128
KH = 2                       # K halves (ICH = 2*128)
CG = 8                       # c's per GEMM/DMA group
# AR batching groups: first group small so routing starts early; at most
# 6 chunks' u in flight at any point (SBUF budget)
GROUPS = [[0], [1, 2], [3, 4, 5], [6, 7]]

FP32 = mybir.dt.float32
FP16 = mybir.dt.float16
ADD = mybir.AluOpType.add
MULT = mybir.AluOpType.mult
SUB = mybir.AluOpType.subtract
AF = mybir.ActivationFunctionType
AX = mybir.AxisListType

LAST_EXEC_NS = None


def _install_profile_hook():
    """antenv.axon_hooks is absent in this image; recreate it so
    run_bass_kernel_spmd(trace=True)/BASS_TRACE can report exec_time_ns."""
    if "antenv.axon_hooks" in sys.modules:
        return
    mod = types.ModuleType("antenv.axon_hooks")
    mod._hook = None
    mod.set_axon_ntff_profile_hook = lambda h: setattr(mod, "_hook", h)
    mod.get_axon_ntff_profile_hook = lambda: mod._hook
    sys.modules["antenv.axon_hooks"] = mod
    try:
        from trn_agent_boot.trn_boot import _ntff_profile_via_ctypes

        hook = _ntff_profile_via_ctypes("/opt/axon/libaxon_pjrt.so")
        if hook is not None:
            mod._hook = hook
    except Exception:
        pass


def _split_sync_waits(nc, max_waits=1):
    """walrus setupSyncWait rejects instructions with more than one sem
    wait; hoist extras onto same-engine InstNoOp's placed just before."""
    for f in nc.m.functions:
        for bb in f.blocks:
            out = []
            changed = False
            for inst in bb.instructions:
                si = inst.sync_info
                waits = list(si.on_wait) if si is not None and si.on_wait else []
                if len(waits) > max_waits:
                    extra, keep = waits[:-max_waits], waits[-max_waits:]
                    for g, w in enumerate(extra):
                        out.append(
                            mybir.InstNoOp(
                                name=f"{inst.name}_wsplit{g}",
                                engine=inst.engine,
                                bass_nofuse=True,
                                sync_info=mybir.SyncInfo(on_wait=[w], on_update=[]),
                            )
                        )
                    inst.sync_info = mybir.SyncInfo(
                        on_wait=keep,
                        on_update=list(si.on_update) if si.on_update else [],
                    )
                    changed = True
                out.append(inst)
            if changed:
                bb.instructions = out


def build_kernel(split_waits=True):
    nc = bass.Bass(
        "TRN2", target_bir_lowering=False, debug=False, num_devices=NCORES
    )
    # x shard: [h, i, bchunk, c, b] fp16
    xt = nc.dram_tensor("xt", [KH, 128, NBCH, CPC, BCH], FP16, kind="ExternalInput").ap()
    # W shard: [h, i, c, (n,o)] fp16
    wt = nc.dram_tensor("wt", [KH, 128, CPC, NCAPS * OCH], FP16, kind="ExternalInput").ap()
    out = nc.dram_tensor("out", [B, NCAPS, OCH], FP32, kind="ExternalOutput").ap()

    # AR staging (per group x iteration); outs in Shared space = fast path
    cc_in, cc_out = {}, {}
    for gi, g in enumerate(GROUPS):
        for it in range(ITERATIONS):
            cc_in[(gi, it)] = nc.dram_tensor(
                f"cc_in_{gi}_{it}", [128, len(g) * NCAPS * OCH], FP32, kind="Internal"
            ).ap()
            cc_out[(gi, it)] = nc.dram_tensor(
                f"cc_out_{gi}_{it}", [128, len(g) * NCAPS * OCH], FP32,
                kind="Internal", addr_space="Shared",
            ).ap()

    with tile.TileContext(nc) as tc:
        with (
            tc.tile_pool(name="xin", bufs=3) as xpool,
            tc.tile_pool(name="psum", bufs=3, space="PSUM") as pspool,
            tc.tile_pool(name="s0ps", bufs=2, space="PSUM") as s0pool,
            tc.tile_pool(name="ubuf", bufs=6) as upool,
            tc.tile_pool(name="vmat", bufs=2) as vpool,
            tc.tile_pool(name="sp", bufs=3) as sppool,
            tc.tile_pool(name="ss", bufs=3) as sspool,
        ):
            # warm up the CC path: the first collective pays ~11us of
            # trigger-start delay; burn it on a dummy 1-element AllReduce
            # long before the first real one is needed.
            warm_in = nc.dram_tensor("cc_warm_i", [128, 1], FP32, kind="Internal").ap()
            warm_out = nc.dram_tensor(
                "cc_warm_o", [128, 1], FP32, kind="Internal", addr_space="Shared"
            ).ap()
            nc.gpsimd.collective_compute(
                "AllReduce",
                ADD,
                replica_groups=[list(range(NCORES))],
                ins=[warm_in[:].opt()],
                outs=[warm_out[:].opt()],
            )

            # resident W: [h] tiles (128i, c*(n o)) fp16
            wsb = {}
            for h in range(KH):
                t = nc.alloc_sbuf_tensor(f"w{h}", [128, CPC * NCAPS * OCH], FP16).ap()
                nc.sync.dma_start(t[:], wt[h].rearrange("i c f -> i (c f)"))
                wsb[h] = t

            # routing scratch (parity-duplicated where cross-chunk overlap matters)
            P = 2
            wn = nc.alloc_sbuf_tensor("wn", [128, NCAPS, OCH, CPC], FP16).ap()
            t32 = nc.alloc_sbuf_tensor("t32", [128, NCAPS, OCH, 2], FP32).ap()
            ty32 = nc.alloc_sbuf_tensor("ty32", [128, NCAPS, 2, CPC], FP32).ap()
            d_all = nc.alloc_sbuf_tensor("d_all", [128, NBCH, CPC], FP32).ap()
            y = [nc.alloc_sbuf_tensor(f"y{p}", [128, NCAPS, CPC], FP32).ap() for p in range(P)]
            dd = [nc.alloc_sbuf_tensor(f"dd{p}", [128, CPC], FP32).ap() for p in range(P)]
            coeff = [nc.alloc_sbuf_tensor(f"coeff{p}", [128, NCAPS, CPC], FP16).ap() for p in range(3)]
            sq = [nc.alloc_sbuf_tensor(f"sq{p}", [128, NCAPS], FP32).ap() for p in range(P)]
            rr = [nc.alloc_sbuf_tensor(f"rr{p}", [128, NCAPS], FP32).ap() for p in range(P)]
            den = [nc.alloc_sbuf_tensor(f"den{p}", [128, NCAPS], FP32).ap() for p in range(P)]
            rec = [nc.alloc_sbuf_tensor(f"rec{p}", [128, NCAPS], FP32).ap() for p in range(P)]
            mmv = [nc.alloc_sbuf_tensor(f"mmv{p}", [128, NCAPS], FP32).ap() for p in range(P)]
            v = [nc.alloc_sbuf_tensor(f"v{p}", [128, NCAPS, OCH], FP32).ap() for p in range(P)]
            sqd = [nc.alloc_sbuf_tensor(f"sqd{p}", [128, OCH], FP32).ap() for p in range(P)]

            def gemm_chunk(bk, gi, k):
                """single-term fp16 GEMM for chunk bk -> u [128, n, o, c] fp16.
                Also accumulates s0_partial = sum_c u on the PE (extra matmuls
                into one PSUM tile) and eagerly stages it into cc_in[(gi,0)],
                so iteration 0 needs no DVE reduction."""
                u = upool.tile([128, NCAPS, OCH, CPC], FP16, tag="u")
                s0 = s0pool.tile([BCH, NCAPS * OCH], FP32, tag="s0")
                for cg in range(CPC // CG):
                    c0 = cg * CG
                    xts = {}
                    for h in range(KH):
                        t = xpool.tile([128, CG, BCH], FP16, tag=f"x{h}")
                        nc.sync.dma_start(t[:], xt[h, :, bk, c0 : c0 + CG, :])
                        xts[h] = t
                    pg = pspool.tile([BCH, CG, NCAPS * OCH], FP32, tag="pg")
                    for j in range(CG):
                        c = c0 + j
                        for h in range(KH):
                            w_sl = wsb[h][:, c * NCAPS * OCH : (c + 1) * NCAPS * OCH]
                            nc.tensor.matmul(
                                pg[:, j, :],
                                lhsT=xts[h][:, j, :],
                                rhs=w_sl,
                                start=(h == 0),
                                stop=(h == KH - 1),
                            )
                            nc.tensor.matmul(
                                s0[:],
                                lhsT=xts[h][:, j, :],
                                rhs=w_sl,
                                start=(c == 0 and h == 0),
                                stop=(c == CPC - 1 and h == KH - 1),
                            )
                    # PSUM (b, cg, (n o)) -> u[b, n, o, c0:c0+8] fp16 strided ACT copy
                    nc.scalar.copy(
                        u[:, :, :, c0 : c0 + CG],
                        pg[:].rearrange("b c (n o) -> b n o c", n=NCAPS),
                    )
                s0sb = sppool.tile([128, NCAPS * OCH], FP32, tag="s0sb")
                nc.scalar.copy(s0sb[:], s0[:])
                nc.sync.dma_start(
                    cc_in[(gi, 0)][:, k * NCAPS * OCH : (k + 1) * NCAPS * OCH],
                    s0sb[:],
                )
                return u

            def tree_sum_c(src, dst_sp):
                """sum over innermost c (64): TT halving tree (fp16 2x_1p),
                fp32 from c=2. src: [128, n, o, 64] fp16; dst_sp fp32."""
                nc.vector.tensor_tensor(
                    wn[:, :, :, :32], src[:, :, :, :32], src[:, :, :, 32:], op=ADD
                )
                for half in (16, 8, 4):
                    nc.vector.tensor_tensor(
                        wn[:, :, :, :half], wn[:, :, :, :half],
                        wn[:, :, :, half : 2 * half], op=ADD,
                    )
                nc.vector.tensor_tensor(
                    t32[:], wn[:, :, :, :2], wn[:, :, :, 2:4], op=ADD
                )
                nc.vector.tensor_tensor(
                    dst_sp.unsqueeze(3), t32[:, :, :, :1], t32[:, :, :, 1:], op=ADD
                )

            def tree_sum_o(dst_y):
                """sum wn over o (middle dim, 64): TT halving tree, fp32 tail.
                wn: [128, n, o, c] fp16 product; dst_y: [128, n, c] fp32."""
                for half in (32, 16, 8, 4):
                    nc.vector.tensor_tensor(
                        wn[:, :, :half, :], wn[:, :, :half, :],
                        wn[:, :, half : 2 * half, :], op=ADD,
                    )
                nc.vector.tensor_tensor(
                    ty32[:], wn[:, :, :2, :], wn[:, :, 2:4, :], op=ADD
                )
                nc.vector.tensor_tensor(
                    dst_y.unsqueeze(2), ty32[:, :, :1, :], ty32[:, :, 1:, :], op=ADD
                )

            def s_partials(gi, it, us, coeffs):
                """per-chunk weighted sums over local c + stage into cc_in.
                (it=0 partials were already staged eagerly by gemm_chunk.)"""
                g = GROUPS[gi]
                bi = cc_in[(gi, it)]
                for k, bk in enumerate(g):
                    spt = sppool.tile([128, NCAPS, OCH], FP32, tag="sp")
                    # coeff [128,n,c] -> per-n view [128,o,c]: o stride-0,
                    # innermost c stride 1 (2x_1p ok)
                    for n in range(NCAPS):
                        cb = coeffs[bk][:, n].unsqueeze(1).broadcast_to(
                            (128, OCH, CPC)
                        )
                        nc.vector.tensor_tensor(
                            wn[:, n], us[bk][:, n], cb, op=MULT
                        )
                    tree_sum_c(wn[:], spt[:])
                    nc.sync.dma_start(
                        bi[:, k * NCAPS * OCH : (k + 1) * NCAPS * OCH],
                        spt[:].rearrange("p n o -> p (n o)"),
                    )

            def group_ar(gi, it):
                g = GROUPS[gi]
                nc.gpsimd.collective_compute(
                    "AllReduce",
                    ADD,
                    replica_groups=[list(range(NCORES))],
                    ins=[cc_in[(gi, it)][:].opt()],
                    outs=[cc_out[(gi, it)][:].opt()],
                )
                ss = sspool.tile([128, len(g), NCAPS, OCH], FP32, tag="ss")
                nc.sync.dma_start(
                    ss[:].rearrange("p g n o -> p (g n o)"), cc_out[(gi, it)][:]
                )
                return ss

            def post_ar(gi, it, ss, us, coeffs_next):
                """squash (mostly ACT), then (if not last iter) y-pass, d update,
                next coeffs. Returns nothing; final iter writes output."""
                g = GROUPS[gi]
                for k, bk in enumerate(g):
                    p = bk % 2
                    sv = ss[:, k]  # [128, n, o] fp32
                    # |s|^2 row sums on ACT (Square + accum_out)
                    for n in range(NCAPS):
                        nc.scalar.activation(
                            sqd[p][:], sv[:, n], AF.Square,
                            accum_out=sq[p][:, n].unsqueeze(1),
                        )
                    if it == 0:
                        # s_true = 0.5*ss: |s|^2 = 0.25*sq
                        nc.vector.tensor_scalar(sq[p][:], sq[p][:], 0.25, None, op0=MULT)
                    nc.scalar.activation(rr[p][:], sq[p][:], AF.Sqrt)
                    nc.vector.scalar_tensor_tensor(
                        den[p][:], sq[p][:], 1.0, rr[p][:], op0=ADD, op1=MULT
                    )
                    nc.vector.reciprocal(rec[p][:], den[p][:])
                    nc.vector.tensor_tensor(mmv[p][:], sq[p][:], rec[p][:], op=MULT)
                    if it == 0:
                        nc.vector.tensor_scalar(mmv[p][:], mmv[p][:], 0.5, None, op0=MULT)
                    # v = s * mmv  (per-partition ACT scale), fp32
                    for n in range(NCAPS):
                        nc.scalar.activation(
                            v[p][:, n], sv[:, n], AF.Copy,
                            scale=mmv[p][:, n].unsqueeze(1),
                        )
                    if it == ITERATIONS - 1:
                        nc.sync.dma_start(
                            out[bk * BCH : (bk + 1) * BCH, :, :], v[p][:]
                        )
                        continue
                    # v_mat: broadcast v over c on ACT, fp16
                    vm = vpool.tile([128, NCAPS, OCH, CPC], FP16, tag="vm")
                    nc.scalar.copy(
                        vm[:],
                        v[p][:].unsqueeze(3).broadcast_to((128, NCAPS, OCH, CPC)),
                    )
                    # y-pass: product + tree over o
                    nc.vector.tensor_tensor(wn[:], us[bk][:], vm[:], op=MULT)
                    tree_sum_o(y[p][:])
                    # d update
                    d = d_all[:, bk, :]
                    nc.vector.tensor_tensor(dd[p][:], y[p][:, 0, :], y[p][:, 1, :], op=SUB)
                    if it == 0:
                        nc.vector.tensor_copy(d, dd[p][:])
                    else:
                        nc.vector.tensor_tensor(d, d, dd[p][:], op=ADD)
                    # coeff for next iteration (fp16, on ACT)
                    cf = coeffs_next[k]
                    nc.scalar.activation(cf[:, 0, :], d, AF.Sigmoid)
                    nc.scalar.activation(cf[:, 1, :], d, AF.Sigmoid, scale=-1.0)

            # ---- schedule ----
            us = {}
            ssbuf = {}

            def emit_gemm_group(gi):
                for k, bk in enumerate(GROUPS[gi]):
                    us[bk] = gemm_chunk(bk, gi, k)

            def cf(gi):
                return {bk: coeff[k] for k, bk in enumerate(GROUPS[gi])}

            def R(gi, it):
                """one pipeline block: consume AR(gi,it); if not last iter,
                emit next partials + AR."""
                post_ar(gi, it, ssbuf[(gi, it)], us, coeff)
                if it < ITERATIONS - 1:
                    s_partials(gi, it + 1, us, cf(gi))
                    ssbuf[(gi, it + 1)] = group_ar(gi, it + 1)

            # G0(1) G1(2) G2(3) = 6 chunks resident; G3(2) reuses G0+G1 bufs
            emit_gemm_group(0)
            ssbuf[(0, 0)] = group_ar(0, 0)
            emit_gemm_group(1)
            ssbuf[(1, 0)] = group_ar(1, 0)
            emit_gemm_group(2)
            ssbuf[(2, 0)] = group_ar(2, 0)
            R(0, 0)
            R(1, 0)
            R(0, 1)            # G0's u freed after this block's s-products
            emit_gemm_group(3)  # AR(3,0) deferred so its input dep can't
            R(1, 1)             # stall later triggers on the CC stream
            R(2, 0)
            ssbuf[(3, 0)] = group_ar(3, 0)
            R(0, 2)
            R(2, 1)
            R(1, 2)
            R(3, 0)
            R(2, 2)
            R(3, 1)
            R(3, 2)

    if split_waits:
        _split_sync_waits(nc)
    return nc


def _prep_inputs(x, W):
    x = np.ascontiguousarray(x, dtype=np.float32)
    W0 = np.ascontiguousarray(W.reshape(NCAPS, C, OCH, ICH), dtype=np.float32)
    xt_cores, wt_cores = [], []
    for k in range(NCORES):
        cs = k * CPC
        xc = x[:, cs : cs + CPC, :]  # (B, 64, 256)
        x6 = xc.reshape(NBCH, BCH, CPC, KH, 128)
        xt = np.ascontiguousarray(x6.transpose(3, 4, 0, 2, 1)).astype(np.float16)
        xt_cores.append(xt)
        Wc = W0[:, cs : cs + CPC]  # (2, 64, 64, 256)
        w5 = Wc.reshape(NCAPS, CPC, OCH, KH, 128)
        wtc = np.ascontiguousarray(w5.transpose(3, 4, 1, 0, 2)).reshape(
            KH, 128, CPC, NCAPS * OCH
        ).astype(np.float16)
        wt_cores.append(wtc)
    return xt_cores, wt_cores


_NC_CACHE = {}


def kernel(x, W):
    global LAST_EXEC_NS
    _install_profile_hook()
    if "nc" not in _NC_CACHE:
        _NC_CACHE["nc"] = build_kernel()
    nc = _NC_CACHE["nc"]
    xt, wt = _prep_inputs(np.asarray(x), np.asarray(W))
    in_maps = [{"xt": xt[k], "wt": wt[k]} for k in range(NCORES)]
    trace = bool(os.environ.get("CAPS_TRACE"))
    res = run_bass_kernel_spmd(nc, in_maps, list(range(NCORES)), trace=trace)
    LAST_EXEC_NS = res.exec_time_ns
    return res.results[0]["out"].astype(np.float32)
